# revision 1
# baseline (speedup 1.0000x reference)
"""Multi-head attention (B=4, S=2048, D=1024, H=16, Hd=64) on 8 NeuronCores.

Sharding: tensor-parallel over heads. Core c owns heads {2c, 2c+1}, i.e. a
128-column slice of Wq/Wk/Wv and the matching 128-row slice of Wo. Each core
computes a full-shape partial output (its heads' contribution through the out
projection); the host sums the 8 partials.

Host-side prep is layout/dtype only (transpose to [D, B*S], cast bf16, slice
weights) plus two exact bias identities:
  * softmax rows sum to 1, so bv contributes exactly (bv @ Wo) to every
    output row -> added on host.
  * bk shifts all scores of a row equally and cancels in softmax; it is
    still applied on-device (free during K-projection eviction).

Device algorithm per core (all matmuls bf16 with f32 PSUM accumulation):
  1. QT = (Wq_c^T q^T + bq_c), KT likewise  -> SBUF [128=d', 8192=s] bf16
     V   = v @ Wv_c                          -> SBUF [s, d'] bf16, stored as
     64 chunks of [128, 130]: cols 0:64 head0-V, 64:66 ones, 66:130 head1-V.
  2. Per (batch, q-slice of 512), interleaved over the 2 heads:
     scores^T tile = K_h Q_h^T (K=64 matmul, 2-head row-tiled on the PE),
     P^T = exp(scores^T / 8) on ScalarE (no max subtraction needed: scores
     are ~N(0,1), nowhere near f32 overflow),
     O^T accumulation: lhsT = [V_h | ones] so the PSUM picks up both the
     attention numerator rows and the softmax row-sum row in one matmul.
     Normalize: reciprocal of the row-sum row, GPSIMD partition-broadcast,
     DVE multiply -> OT SBUF [128=d', 8192=s] bf16.
  3. out_partial = OT^T @ Wo_c per s-tile -> DRAM f32.
"""

import os
from contextlib import ExitStack

import numpy as np
import ml_dtypes

import concourse.bass as bass
import concourse.mybir as mybir
import concourse.tile as tile
from concourse import bacc, library_config
from concourse.bass_utils import run_bass_kernel_spmd

B, S, D, H, HD = 4, 2048, 1024, 16, 64
BS = B * S                     # 8192 flattened tokens
NCORES = 8
HPC = H // NCORES              # 2 heads per core
DC = HPC * HD                  # 128-wide weight slice per core

F32 = mybir.dt.float32
BF16 = mybir.dt.bfloat16
EXP = mybir.ActivationFunctionType.Exp

_BUILT = None                  # (nc, tmpdir) cache — compile once per process
LAST_EXEC_NS = None
LAST_RESULTS = None


def _build_program():
    nc = bacc.Bacc("TRN2", target_bir_lowering=False, debug=False,
                   num_devices=NCORES)

    qT_d = nc.dram_tensor("qT", [D, BS], BF16, kind="ExternalInput").ap()
    kT_d = nc.dram_tensor("kT", [D, BS], BF16, kind="ExternalInput").ap()
    vT_d = nc.dram_tensor("vT", [D, BS], BF16, kind="ExternalInput").ap()
    wq_d = nc.dram_tensor("wq", [D, DC], BF16, kind="ExternalInput").ap()
    wk_d = nc.dram_tensor("wk", [D, DC], BF16, kind="ExternalInput").ap()
    wv_d = nc.dram_tensor("wv", [D, DC], BF16, kind="ExternalInput").ap()
    wo_d = nc.dram_tensor("wo", [DC, D], BF16, kind="ExternalInput").ap()
    bq_d = nc.dram_tensor("bq", [DC, 1], F32, kind="ExternalInput").ap()
    bk_d = nc.dram_tensor("bk", [DC, 1], F32, kind="ExternalInput").ap()
    out_d = nc.dram_tensor("out", [BS, D], F32, kind="ExternalOutput").ap()

    with tile.TileContext(nc) as tc, ExitStack() as ctx:
        const = ctx.enter_context(tc.tile_pool(name="const", bufs=1))
        persist = ctx.enter_context(tc.tile_pool(name="persist", bufs=1))
        stage = ctx.enter_context(tc.tile_pool(name="stage", bufs=3))
        ptpool = ctx.enter_context(tc.tile_pool(name="ptpool", bufs=6))
        npool = ctx.enter_context(tc.tile_pool(name="npool", bufs=3))
        ostage = ctx.enter_context(tc.tile_pool(name="ostage", bufs=3))
        # PSUM: "big" 3 slots x 2 banks (scores + all transient accумulators)
        #       "pop" 2 slots x 1 bank (the two per-head O^T accumulators)
        big = ctx.enter_context(tc.tile_pool(name="big", bufs=3, space="PSUM"))
        pop = ctx.enter_context(tc.tile_pool(name="pop", bufs=2, space="PSUM"))

        # ---- persistent SBUF state -------------------------------------
        QT = persist.tile([128, BS], BF16)          # [d' , s]
        KT = persist.tile([128, BS], BF16)
        OT = persist.tile([128, BS], BF16)
        # V extended, per 128-token chunk (free layout [2, 132], abs width 264):
        #   abs cols 0:64    = V_h0          (h0 lhsT = abs 0:65, rsum row 64)
        #   abs col  64      = ones
        #   abs col  68      = ones          (h1 lhsT = abs 68:196, rsum row 0)
        #   abs cols 132:196 = V_h1          (-> h1 lhsT rows 64:128)
        #   everything else zero (h1 lhsT rows 1:64 are garbage, never read)
        VE = persist.tile([128, 64, 2, 132], BF16)

        # ---- constants --------------------------------------------------
        wq_sb = const.tile([128, 8, DC], BF16)
        wk_sb = const.tile([128, 8, DC], BF16)
        wv_sb = const.tile([128, 8, DC], BF16)
        wo_sb = const.tile([128, D], BF16)
        bq_sb = const.tile([128, 1], F32)
        bk_sb = const.tile([128, 1], F32)
        ones_sb = const.tile([128, 64], F32)
        nc.vector.memset(ones_sb[:], 1.0)
        nc.sync.dma_start(wq_sb[:], wq_d.rearrange("(c p) d -> p c d", p=128))
        nc.sync.dma_start(wk_sb[:], wk_d.rearrange("(c p) d -> p c d", p=128))
        nc.sync.dma_start(wv_sb[:], wv_d.rearrange("(c p) d -> p c d", p=128))
        nc.sync.dma_start(wo_sb[:], wo_d)
        nc.sync.dma_start(bq_sb[:], bq_d)
        nc.sync.dma_start(bk_sb[:], bk_d)
        nc.vector.memset(VE[:], 0.0)
        nc.vector.memset(VE[:, :, 0, 64:65], 1.0)
        nc.vector.memset(VE[:, :, 0, 68:69], 1.0)

        # ---- phase 1: projections --------------------------------------
        for srcT, w_sb, b_sb, dstT in (
            (qT_d, wq_sb, bq_sb, QT),
            (kT_d, wk_sb, bk_sb, KT),
        ):
            for ss in range(16):                    # s-slices of 512
                xt = stage.tile([128, 8, 512], BF16, tag="xT")
                nc.sync.dma_start(
                    xt[:],
                    srcT.rearrange("(c p) s -> p c s", p=128)[
                        :, :, ss * 512:(ss + 1) * 512],
                )
                ps = big.tile([128, 512], F32, tag="sc", name="psqk")
                for c in range(8):
                    nc.tensor.matmul(ps[:], lhsT=w_sb[:, c], rhs=xt[:, c],
                                     start=(c == 0), stop=(c == 7))
                nc.vector.tensor_scalar_add(
                    dstT[:, ss * 512:(ss + 1) * 512], ps[:], b_sb[:])

        for ss in range(16):
            vt = stage.tile([128, 8, 512], BF16, tag="xT")
            nc.sync.dma_start(
                vt[:],
                vT_d.rearrange("(c p) s -> p c s", p=128)[
                    :, :, ss * 512:(ss + 1) * 512],
            )
            for st in range(4):                     # s-tiles of 128
                chunk = ss * 4 + st
                ps = big.tile([128, 512], F32, tag="sc", name="psv")
                for c in range(8):
                    nc.tensor.matmul(
                        ps[:, 0:DC],
                        lhsT=vt[:, c, st * 128:(st + 1) * 128],
                        rhs=wv_sb[:, c],
                        start=(c == 0), stop=(c == 7))
                # h0 cols 0:64 -> VE[..,0,0:64]; h1 cols 64:128 -> VE[..,1,0:64]
                dst = VE[:, chunk, :, 0:64]
                src = ps[:, 0:DC].rearrange("p (a x) -> p a x", a=2)
                nc.vector.tensor_copy(dst, src)

        # ---- phase 2+3: attention, then out-proj, per batch -------------
        for b in range(B):
            for qs in range(4):                     # q-slices of 512
                q0 = b * S + qs * 512
                po = [pop.tile([128, 512], F32, tag="po", name=f"po{h}")
                      for h in range(HPC)]
                def emit_pv(kg, pt):
                    # O^T accumulation for k-group kg, both heads
                    for h in range(HPC):
                        for kt2 in range(2):
                            chunk = b * 16 + kg * 2 + kt2
                            first = (kg == 0 and kt2 == 0)
                            last = (kg == 7 and kt2 == 1)
                            ve_flat = VE[:, chunk, :, :].rearrange(
                                "p a x -> p (a x)")
                            if h == 0:
                                # rows 0:64 = O^T_h0, row 64 = rowsum_h0
                                nc.tensor.matmul(
                                    po[0][0:65, :],
                                    lhsT=ve_flat[:, 0:65],
                                    rhs=pt[0][:, kt2],
                                    start=first, stop=last)
                            else:
                                # abs cols 68:196: row 0 = rowsum_h1 (ones
                                # at abs 68), rows 64:128 = O^T_h1 (V_h1)
                                nc.tensor.matmul(
                                    po[1][:, :],
                                    lhsT=ve_flat[:, 68:196],
                                    rhs=pt[1][:, kt2],
                                    start=first, stop=last)

                # Software pipeline: PV for k-group kg-1 is emitted between
                # QK(kg) and QK(kg+1), so the PE never sits waiting on the
                # ScalarE exp of the k-group it just produced.
                prev_pt = None
                for kg in range(8):                 # k-groups of 2x128
                    pt = []
                    for h in range(HPC):
                        psc = big.tile([128, 2, 512], F32, tag="sc", name="psc")
                        for kt2 in range(2):
                            k0 = b * S + (kg * 2 + kt2) * 128
                            nc.tensor.matmul(
                                psc[:, kt2],
                                lhsT=KT[64 * h:64 * h + 64, k0:k0 + 128],
                                rhs=QT[64 * h:64 * h + 64, q0:q0 + 512],
                                start=True, stop=True)
                        p = ptpool.tile([128, 2, 512], BF16, tag="pt")
                        nc.scalar.activation(
                            p.rearrange("p a x -> p (a x)"),
                            psc.rearrange("p a x -> p (a x)"),
                            EXP, scale=0.125)
                        pt.append(p)
                    if prev_pt is not None:
                        emit_pv(kg - 1, prev_pt)
                    prev_pt = pt
                emit_pv(7, prev_pt)
                # normalize -> OT. The per-q reciprocal rowsum lives on one
                # partition; replicate it across the head's 64 partitions
                # with a K=1 PE matmul (ones column x reciprocal row).
                for h in range(HPC):
                    rs = 64 if h == 0 else 0        # rowsum partition
                    d0 = 64 * h                     # head's partition base
                    rr = npool.tile([128, 512], F32, tag="rr")
                    nc.vector.reciprocal(rr[rs:rs + 1, :], po[h][rs:rs + 1, :])
                    bcp = big.tile([128, 512], F32, tag="sc", name="bcp")
                    nc.tensor.matmul(
                        bcp[d0:d0 + 64, :],
                        lhsT=ones_sb[rs:rs + 1, :],
                        rhs=rr[rs:rs + 1, :],
                        start=True, stop=True)
                    bc = npool.tile([128, 512], F32, tag="bc")
                    nc.vector.tensor_copy(bc[d0:d0 + 64, :], bcp[d0:d0 + 64, :])
                    osrc = po[h][0:64, :] if h == 0 else po[h][64:128, :]
                    nc.vector.tensor_mul(
                        OT[d0:d0 + 64, q0:q0 + 512], osrc, bc[d0:d0 + 64, :])

            # out-projection for this batch's s-range
            for st in range(16):
                s0 = b * S + st * 128
                osb = ostage.tile([128, D], F32, tag="osb")
                for ns in range(2):
                    ps3 = big.tile([128, 512], F32, tag="sc", name="ps3")
                    nc.tensor.matmul(
                        ps3[:],
                        lhsT=OT[:, s0:s0 + 128],
                        rhs=wo_sb[:, ns * 512:(ns + 1) * 512],
                        start=True, stop=True)
                    if ns == 0:
                        nc.vector.tensor_copy(osb[:, 0:512], ps3[:])
                    else:
                        nc.scalar.copy(osb[:, 512:1024], ps3[:])
                nc.sync.dma_start(out_d[s0:s0 + 128, :], osb[:])

    nc.compile()
    return nc


def _get_program():
    global _BUILT
    if _BUILT is None:
        _BUILT = _build_program()
    return _BUILT


def kernel(q, k, v, Wq, bq, Wk, bk, Wv, bv, Wo, bo, trace=None):
    global LAST_EXEC_NS, LAST_RESULTS
    if trace is None:
        trace = os.environ.get("KERNEL_TRACE", "0") == "1"
    bf16 = ml_dtypes.bfloat16

    q2 = np.asarray(q, np.float32).reshape(BS, D)
    k2 = np.asarray(k, np.float32).reshape(BS, D)
    v2 = np.asarray(v, np.float32).reshape(BS, D)
    qT = np.ascontiguousarray(q2.T).astype(bf16)
    kT = np.ascontiguousarray(k2.T).astype(bf16)
    vT = np.ascontiguousarray(v2.T).astype(bf16)

    Wq = np.asarray(Wq, np.float32)
    Wk = np.asarray(Wk, np.float32)
    Wv = np.asarray(Wv, np.float32)
    Wo = np.asarray(Wo, np.float32)
    bq = np.asarray(bq, np.float32)
    bk = np.asarray(bk, np.float32)
    bv = np.asarray(bv, np.float32)
    bo = np.asarray(bo, np.float32)

    in_maps = []
    for c in range(NCORES):
        sl = slice(c * DC, (c + 1) * DC)
        in_maps.append({
            "qT": qT, "kT": kT, "vT": vT,
            "wq": np.ascontiguousarray(Wq[:, sl]).astype(bf16),
            "wk": np.ascontiguousarray(Wk[:, sl]).astype(bf16),
            "wv": np.ascontiguousarray(Wv[:, sl]).astype(bf16),
            "wo": np.ascontiguousarray(Wo[sl, :]).astype(bf16),
            "bq": np.ascontiguousarray(bq[sl]).reshape(DC, 1),
            "bk": np.ascontiguousarray(bk[sl]).reshape(DC, 1),
        })

    nc = _get_program()
    res = run_bass_kernel_spmd(nc, in_maps, list(range(NCORES)), trace=trace)
    LAST_EXEC_NS = res.exec_time_ns
    LAST_RESULTS = res

    out = np.zeros((BS, D), np.float32)
    for c in range(NCORES):
        out += np.asarray(res.results[c]["out"], np.float32)
    out += bv.astype(np.float32) @ Wo + bo          # exact bias identities
    return out.reshape(B, S, D)



# revision 37
# speedup vs baseline: 1.4738x; 1.4738x over previous
"""Multi-head attention (B=4, S=2048, D=1024, H=16, Hd=64) on 8 NeuronCores.

Sharding: batch x head-group. Core c owns batch b=c//2 and head-group
g2=c%2 (8 heads = a 512-wide slice of Wq/Wk/Wv columns and Wo rows). Each
core reads only its batch's q/k/v (bf16) and emits a full-width
[2048, 1024] f32 partial; the host sums the two partials per batch.

Exact host-side bias identities: softmax rows sum to 1 so bv contributes
exactly bv @ Wo (host adds it, with bo); bq/bk applied on-device during
projection eviction. All matmuls bf16 with f32 PSUM (fp8 was tried and
measured at ~11% L2 error: quantization error of a random-sign operand
passes ~1:1 into the output, so every attention operand must be bf16).

Device algorithm per core:
  1. Projections (bf16, 8x128-contraction chunks): K^T, Q^T -> SBUF
     [128, hg, s] (partition 64*(h%2)+dim, plane hg=h//2); V -> SBUF
     [128=tok%128, ktile, h//2, h%2, col] with a ones column (even heads
     col 64 of [V|1]; odd heads col 0 of [1|0|V]) so PV also produces the
     softmax row-sum, and odd heads land on PSUM partitions 64:128.
  2. Attention, head-pipelined: per (q-slice 512, head): 6 score/exp
     groups (ktiles 3+3+3+3+2+2; exp reads 1536/1024-wide PSUM spans to
     amortize ScalarE overhead -- ScalarE does 33.5M exps/core and is
     near-roofline). exp(s/8 - 1) -> bf16 P tiles (the -1 cancels in
     normalization). PV for the previous head is interleaved between
     groups; K/Q/V projection leftovers and the out-projection ride as
     side work, one PSUM tile per group, so the PE never idles long.
     Normalize: DVE rowsum copy, GPSIMD partition-broadcast, DVE fast
     approximate reciprocal + multiply -> OT bf16.
  3. Out-projection: per 128-token tile, 4-chunk bf16 accumulation
     against Wo -> f32 partial out, interleaved into the next q-slice.
"""

import os
from contextlib import ExitStack

import numpy as np
import ml_dtypes

import concourse.bass as bass
import concourse.mybir as mybir
import concourse.tile as tile
from concourse import bacc
from concourse.bass_utils import run_bass_kernel_spmd

B, S, D, H, HD = 4, 2048, 1024, 16, 64
NCORES = 8
HPC = 8                       # heads per core
DC = HPC * HD                 # 512-wide weight slice per core

F32 = mybir.dt.float32
BF16 = mybir.dt.bfloat16
EXP = mybir.ActivationFunctionType.Exp

EXP_BIAS = -1.0               # exp(s/8 - 1); cancels in the normalization
FILL = 0                      # optional redundant score matmuls per group

_BUILT = None
LAST_EXEC_NS = None
LAST_RESULTS = None

GROUPS = (3, 3, 3, 3, 2, 2)   # ktiles per exp group (16 total)
DEBUG = os.environ.get("KERNEL_DEBUG", "0") == "1"


def _build_program():
    nc = bacc.Bacc("TRN2", target_bir_lowering=False, debug=False,
                   num_devices=NCORES)

    # DRAM I/O. x layout [p, (c, s)]: input dim d = c*128+p.
    q_d = nc.dram_tensor("qx", [128, 8, 2048], BF16, kind="ExternalInput").ap()
    k_d = nc.dram_tensor("kx", [128, 8, 2048], BF16, kind="ExternalInput").ap()
    v_d = nc.dram_tensor("vx", [128, 8, 2048], BF16, kind="ExternalInput").ap()
    wq_d = nc.dram_tensor("wq", [128, 4096], BF16, kind="ExternalInput").ap()
    wk_d = nc.dram_tensor("wk", [128, 4096], BF16, kind="ExternalInput").ap()
    wv_d = nc.dram_tensor("wv", [128, 4096], BF16, kind="ExternalInput").ap()
    wo_d = nc.dram_tensor("wo", [128, 4096], BF16, kind="ExternalInput").ap()
    bqk_d = nc.dram_tensor("bqk", [128, 8], F32, kind="ExternalInput").ap()
    out_d = nc.dram_tensor("out", [S, D], F32, kind="ExternalOutput").ap()

    with tile.TileContext(nc) as tc, ExitStack() as ctx:
        const = ctx.enter_context(tc.tile_pool(name="const", bufs=1))
        persist = ctx.enter_context(tc.tile_pool(name="persist", bufs=1))
        xstage = ctx.enter_context(tc.tile_pool(name="xstage", bufs=3))
        p8pool = ctx.enter_context(tc.tile_pool(name="p8pool", bufs=2))
        rrpool = ctx.enter_context(tc.tile_pool(name="rrpool", bufs=2))
        bcpool = ctx.enter_context(tc.tile_pool(name="bcpool", bufs=2))
        osbpool = ctx.enter_context(tc.tile_pool(name="osbpool", bufs=2))
        # PSUM: scores 2x3 banks, PV accumulator 1, misc (proj/outproj) 1
        scp = ctx.enter_context(tc.tile_pool(name="scp", bufs=2, space="PSUM"))
        pop = ctx.enter_context(tc.tile_pool(name="pop", bufs=1, space="PSUM"))
        mip = ctx.enter_context(tc.tile_pool(name="mip", bufs=1, space="PSUM"))

        # ---- constants / persistent state --------------------------------
        wqs = const.tile([128, 8, 512], BF16)
        wks = const.tile([128, 8, 512], BF16)
        wvs = const.tile([128, 8, 512], BF16)
        wos = const.tile([128, 4, 1024], BF16)
        bqk = const.tile([128, 8], F32)
        ebias = const.tile([128, 1], F32)
        ones = const.tile([128, 128], BF16)
        nc.vector.memset(ebias[:], EXP_BIAS)
        nc.vector.memset(ones[:], 1.0)

        K8 = persist.tile([128, 4, 2048], BF16)      # [64*(h%2)+d, h//2, s]
        Q8 = persist.tile([128, 4, 2048], BF16)
        # [tok%128, ktile, h//2, h%2, col]
        V8 = persist.tile([128, 16, 4, 2, 128], BF16)
        OT = persist.tile([128, 4, 2048], BF16)      # [64*(h%2)+d, h//2, s]

        nc.sync.dma_start(wks[:].rearrange("p a j -> p (a j)"), wk_d)
        nc.sync.dma_start(wqs[:].rearrange("p a j -> p (a j)"), wq_d)
        nc.sync.dma_start(bqk[:], bqk_d)
        nc.sync.dma_start(wvs[:].rearrange("p a j -> p (a j)"), wv_d)
        nc.sync.dma_start(wos[:].rearrange("p a j -> p (a j)"), wo_d)

        # V8 ones/zeros framing (rowsum rides in the PV matmul):
        #   even h: cols 0:64 = V, col 64 = 1.0 -> po rows 0:65 (base 0)
        #   odd h:  col 0 = 1.0, cols 1:64 = 0, 64:128 = V -> po full bank
        nc.vector.memset(V8[:, :, :, 1, 0:64], 0.0)
        nc.vector.memset(V8[:, :, :, 1, 0:1], 1.0)
        nc.vector.memset(V8[:, :, :, 0, 64:65], 1.0)

        # ---- op emitters -------------------------------------------------
        misc_rot = [0]

        def misc_tile(allow_po):
            misc_rot[0] ^= 1
            if allow_po and misc_rot[0]:
                return pop.tile([128, 512], F32, tag="po", name="mt")
            return mip.tile([128, 512], F32, tag="mt", name="mt")

        xlast = {}

        def xslice(x_d, key, ss):
            # stage [128, 8, 512] input slice (all 8 D-chunks of 512 tokens);
            # only the most recent slice is cached (the pool slots rotate)
            if xlast.get("key") != (key, ss):
                xt = xstage.tile([128, 8, 512], BF16, tag="xs", name="xs")
                nc.sync.dma_start(xt[:], x_d[:, :, ss * 512:(ss + 1) * 512])
                xlast["key"] = (key, ss)
                xlast["tile"] = xt
            return xlast["tile"]

        def qk_proj(dst, w, x_d, key, bcol, cc, ss, allow_po):
            xt = xslice(x_d, key, ss)
            ps = misc_tile(allow_po)
            for c in range(8):
                nc.tensor.matmul(
                    ps[:], lhsT=w[:, c, cc * 128:(cc + 1) * 128],
                    rhs=xt[:, c, :],
                    start=(c == 0), stop=(c == 7))
            nc.vector.tensor_scalar_add(
                dst[:, cc, ss * 512:(ss + 1) * 512], ps[:],
                bqk[:, bcol + cc:bcol + cc + 1])

        def v_proj(st, allow_po):
            xt = xslice(v_d, "v", st // 4)
            ps = misc_tile(allow_po)
            for c in range(8):
                nc.tensor.matmul(
                    ps[:], lhsT=xt[:, c, (st % 4) * 128:(st % 4) * 128 + 128],
                    rhs=wvs[:, c, :],
                    start=(c == 0), stop=(c == 7))
            src = ps[:].rearrange("p (hh par x) -> p hh par x", hh=4, par=2)
            nc.vector.tensor_copy(V8[:, st, :, 0, 0:64], src[:, :, 0, :])
            nc.vector.tensor_copy(V8[:, st, :, 1, 64:128], src[:, :, 1, :])

        def outproj(qs, st, half, allow_po):
            ps = misc_tile(allow_po)
            s0 = qs * 512 + st * 128
            for c in range(4):
                nc.tensor.matmul(
                    ps[:], lhsT=OT[:, c, s0:s0 + 128],
                    rhs=wos[:, c, half * 512:(half + 1) * 512],
                    start=(c == 0), stop=(c == 3))
            if half == 0:
                osb = osbpool.tile([128, 1024], F32, tag="osb", name="osb")
                outproj.osb = osb
            else:
                osb = outproj.osb
            nc.vector.tensor_copy(osb[:, half * 512:(half + 1) * 512], ps[:])
            if half == 1:
                nc.sync.dma_start(out_d[s0:s0 + 128, :], osb[:])

        def emit_pv(pend, kt):
            qs, lh, p8, po = pend
            if kt == 0:
                po = pop.tile([128, 512], F32, tag="po", name="po")
                pend[3] = po
            else:
                po = pend[3]
            hh, par = lh // 2, lh % 2
            if par == 0:
                nc.tensor.matmul(po[0:65, :], lhsT=V8[:, kt, hh, 0, 0:65],
                                 rhs=p8[:, kt, :],
                                 start=(kt == 0), stop=(kt == 15))
            else:
                nc.tensor.matmul(po[0:128, :], lhsT=V8[:, kt, hh, 1, 0:128],
                                 rhs=p8[:, kt, :],
                                 start=(kt == 0), stop=(kt == 15))

        def normalize(pend):
            # po rows bb:bb+64 = unnormalized O^T, row rs = softmax rowsum.
            # Broadcast the rowsum row across all 128 partitions with a K=1
            # bf16 PE matmul; copy + approximate-reciprocal on DVE strictly
            # at partition base 0 (the custom DVE op is broken at base 64);
            # final multiply mixes PSUM in0 (base bb) with SBUF in1 (base 0),
            # which the ISA allows cross-base.
            qs, lh, p8, po = pend
            cc, bb = lh // 2, 64 * (lh % 2)
            rs = 64 if lh % 2 == 0 else 0   # rowsum partition in po
            rr = rrpool.tile([128, 512], BF16, tag="rr", name="rr")
            bc = bcpool.tile([128, 2, 512], F32, tag="bc", name="bc")
            nc.vector.tensor_copy(rr[rs:rs + 1, :], po[rs:rs + 1, :])
            bcp = misc_tile(False)
            nc.tensor.matmul(
                bcp[0:64, :],
                lhsT=ones[rs:rs + 1, 0:64],
                rhs=rr[rs:rs + 1, :],
                start=True, stop=True)
            nc.vector.tensor_copy(bc[0:64, 0, :], bcp[0:64, :])
            nc.vector.reciprocal_approx_fast(
                out=bc[0:64, 1, :], in_=bc[0:64, 0, :])
            nc.vector.tensor_mul(
                OT[bb:bb + 64, cc, qs * 512:(qs + 1) * 512],
                po[bb:bb + 64, :], bc[0:64, 1, :])

        # ---- phase 1: K plane 0, Q (plane 0, slice 0), all of V ----------
        # Heads 0/1 only need K8 plane 0 and Q8[:, 0, 0:512]; PV of head 0
        # (running during head 1) needs all of V8. The rest of K/Q trickles
        # in as side work during early attention.
        for ss in range(4):
            qk_proj(K8, wks, k_d, "k", 4, 0, ss, allow_po=True)
        qk_proj(Q8, wqs, q_d, "q", 0, 0, 0, allow_po=True)
        for st in range(16):
            v_proj(st, allow_po=True)

        # ---- phase 2: attention with pipelined PV / side work ------------
        # side-work order matters: heads 2cc/2cc+1 of q-slice qs consume
        # K plane cc and Q[:, cc, qs]; keep producers several slots ahead.
        side = []
        for cc in range(1, 4):
            for ss in range(4):
                side.append(("k", cc, ss))
            side.append(("q", cc, 0))
        side += [("q", cc, 1) for cc in range(4)]
        pend = None
        for qs in range(4):
            if 0 < qs < 3:
                side += [("q", cc, qs + 1) for cc in range(4)]
            for lh in range(8):
                hg, par = lh // 2, lh % 2
                p8 = p8pool.tile([128, 16, 512], BF16, tag="p8", name="p8")
                cur = [qs, lh, p8, None]
                kt0 = 0
                pv_sched = (3, 3, 3, 3, 2, 2)
                for gi, gsz in enumerate(GROUPS):
                    sc = scp.tile([128, 3, 512], F32, tag="sc", name="sc")
                    for j in [0] * FILL + list(range(gsz)):
                        kt = kt0 + j
                        nc.tensor.matmul(
                            sc[:, j, :],
                            lhsT=K8[64 * par:64 * par + 64, hg,
                                    kt * 128:(kt + 1) * 128],
                            rhs=Q8[64 * par:64 * par + 64, hg,
                                   qs * 512:(qs + 1) * 512],
                            start=True, stop=True)
                    nc.scalar.activation(
                        p8[:, kt0:kt0 + gsz, :], sc[:, 0:gsz, :],
                        EXP, bias=ebias[:], scale=0.125)
                    kt0 += gsz
                    if pend is not None:
                        base = sum(pv_sched[:gi])
                        for kt in range(base, base + pv_sched[gi]):
                            emit_pv(pend, kt)
                    if gi > 0 and side:
                        item = side.pop(0)
                        allow_po = pend is None
                        if item[0] == "k":
                            qk_proj(K8, wks, k_d, "k", 4, item[1], item[2],
                                    allow_po)
                        elif item[0] == "q":
                            qk_proj(Q8, wqs, q_d, "q", 0, item[1], item[2],
                                    allow_po)
                        else:
                            outproj(item[1], item[2], item[3], allow_po)
                if pend is not None:
                    normalize(pend)
                    # qs-1's OT is now fully written (head 7 normalized
                    # above): its out-projection may enter the side queue
                    if qs > 0 and lh == 0:
                        for st in range(4):
                            side.append(("o", qs - 1, st, 0))
                            side.append(("o", qs - 1, st, 1))
                pend = cur
        # tail: PV + normalize of the last head, then the last out-proj
        for kt in range(16):
            emit_pv(pend, kt)
        normalize(pend)
        for st in range(4):
            outproj(3, st, 0, allow_po=True)
            outproj(3, st, 1, allow_po=True)

        if DEBUG:
            dK = nc.dram_tensor("dK", [128, 4 * 2048], BF16,
                                kind="ExternalOutput").ap()
            dQ = nc.dram_tensor("dQ", [128, 4 * 2048], BF16,
                                kind="ExternalOutput").ap()
            dV = nc.dram_tensor("dV", [128, 16 * 4 * 2 * 128], BF16,
                                kind="ExternalOutput").ap()
            dOT = nc.dram_tensor("dOT", [128, 4 * 2048], BF16,
                                 kind="ExternalOutput").ap()
            nc.sync.dma_start(dK, K8[:].rearrange("p a s -> p (a s)"))
            nc.sync.dma_start(dQ, Q8[:].rearrange("p a s -> p (a s)"))
            nc.sync.dma_start(dV, V8[:].rearrange("p a b c d -> p (a b c d)"))
            nc.sync.dma_start(dOT, OT[:].rearrange("p a s -> p (a s)"))

    nc.compile()
    return nc


def _get_program():
    global _BUILT
    if _BUILT is None:
        _BUILT = _build_program()
    return _BUILT


def kernel(q, k, v, Wq, bq, Wk, bk, Wv, bv, Wo, bo, trace=None):
    global LAST_EXEC_NS, LAST_RESULTS
    if trace is None:
        trace = os.environ.get("KERNEL_TRACE", "0") == "1"
    bf16 = ml_dtypes.bfloat16

    q = np.asarray(q, np.float32)
    k = np.asarray(k, np.float32)
    v = np.asarray(v, np.float32)
    Wq = np.asarray(Wq, np.float32)
    Wk = np.asarray(Wk, np.float32)
    Wv = np.asarray(Wv, np.float32)
    Wo = np.asarray(Wo, np.float32)
    bq = np.asarray(bq, np.float32)
    bk = np.asarray(bk, np.float32)
    bv = np.asarray(bv, np.float32)
    bo = np.asarray(bo, np.float32)

    def pack_x(x):   # [S, D] f32 -> [128, 8, 2048] bf16, [p, c, s]
        xt = np.ascontiguousarray(x.T)                      # [1024, 2048]
        xt = xt.reshape(8, 128, S).transpose(1, 0, 2)
        return np.ascontiguousarray(xt).astype(bf16)

    def pack_w(W, g2):   # [D, D] -> [128, 4096] bf16: [p, (c, 512 cols)]
        Wl = W[:, g2 * DC:(g2 + 1) * DC]                    # [1024, 512]
        Wl = Wl.reshape(8, 128, DC).transpose(1, 0, 2)
        return np.ascontiguousarray(Wl).reshape(128, 4096).astype(bf16)

    def pack_wo(W, g2):
        Wl = W[g2 * DC:(g2 + 1) * DC, :]                    # [512, 1024]
        Wl = Wl.reshape(4, 128, D).transpose(1, 0, 2)
        return np.ascontiguousarray(Wl).reshape(128, 4096).astype(bf16)

    def pack_bqk(bq_, bk_, g2):
        out = np.empty((128, 8), np.float32)
        out[:, 0:4] = bq_[g2 * DC:(g2 + 1) * DC].reshape(4, 128).T
        out[:, 4:8] = bk_[g2 * DC:(g2 + 1) * DC].reshape(4, 128).T
        return out

    xq = [pack_x(q[b]) for b in range(B)]
    xk = [pack_x(k[b]) for b in range(B)]
    xv = [pack_x(v[b]) for b in range(B)]
    wqs = [pack_w(Wq, g) for g in range(2)]
    wks = [pack_w(Wk, g) for g in range(2)]
    wvs = [pack_w(Wv, g) for g in range(2)]
    wos = [pack_wo(Wo, g) for g in range(2)]
    bqks = [pack_bqk(bq, bk, g) for g in range(2)]

    in_maps = []
    for c in range(NCORES):
        b, g2 = c // 2, c % 2
        in_maps.append({
            "qx": xq[b], "kx": xk[b], "vx": xv[b],
            "wq": wqs[g2], "wk": wks[g2], "wv": wvs[g2],
            "wo": wos[g2], "bqk": bqks[g2],
        })

    nc = _get_program()
    res = run_bass_kernel_spmd(nc, in_maps, list(range(NCORES)), trace=trace)
    LAST_EXEC_NS = res.exec_time_ns
    LAST_RESULTS = res

    extra = bv @ Wo + bo                       # exact bias identities
    out = np.empty((B, S, D), np.float32)
    for b in range(B):
        out[b] = np.asarray(res.results[2 * b]["out"], np.float32)
        out[b] += np.asarray(res.results[2 * b + 1]["out"], np.float32)
        out[b] += extra
    return out
